# revision 33
# baseline (speedup 1.0000x reference)
"""3-layer GRU (PyTorch gate order) + BatchNorm1d (batch stats) + FC + sigmoid.

Strategy: data-parallel over batch across 8 NeuronCores (64 rows/core),
GRU weights replicated, bf16 matmul operands with fp32 PSUM accumulation.

Per core:
  A0 (upfront): gx0 = x @ Wih0T + bias, written to DRAM (bf16).
  R_l (l=0,1,2): sequential recurrence.  gh = h @ WhhT with lhsT = hT
      pair tiles (bf16, [128, 128] = two timesteps of transposed state),
      rhs = WhhT chunks resident in SBUF.  PSUM split into RZ [64, 2048]
      and N [64, 1024] so the tanh tail doesn't block the next step's
      r/z matmuls.  bhh_n accumulated via K=1 ones-matmul.
      h_new = n + z*(h_prev - n) (saves the 1-z activation).
      Interleaved into R_l's PE stream: phase A of layer l+1
      (gx_{l+1} = h_l @ Wih_{l+1}T + bias) consuming the hT pair tiles
      directly from SBUF, half-blocks emitted between gh and the
      transposes so the PE never idles during the elementwise chain.
      gx ping-pongs between two DRAM buffers (bf16).

BN batch stats: per-core partial sum/sumsq via ones-matmuls -> AllReduce
across the 8 cores -> BN+FC folded into y = h63 @ (gamma*rstd*fcW) + C.
"""

import numpy as np
import ml_dtypes

import concourse.bacc as bacc
import concourse.bass as bass
import concourse.mybir as mybir
import concourse.tile as tile
from concourse.bass_utils import run_bass_kernel_spmd

N_CORES = 8
B, T, F, H = 512, 64, 64, 1024
BL = B // N_CORES          # 64 batch rows per core
G = 3 * H                  # 3072 gates
KH = H // 128              # 8 contraction chunks
EPS = 1e-5

F32 = mybir.dt.float32
BF16 = mybir.dt.bfloat16
AOP = mybir.AluOpType
ACTF = mybir.ActivationFunctionType


def _emit(nc, tc, seq_len):
    T_ = seq_len
    nblocks = T_ * BL // 128   # 128-token (2-step) blocks

    xT = nc.dram_tensor("xT", [F, T_, BL], BF16, kind="ExternalInput").ap()
    wih = [
        nc.dram_tensor("wih0T", [F, G], BF16, kind="ExternalInput").ap(),
        nc.dram_tensor("wih1T", [H, G], BF16, kind="ExternalInput").ap(),
        nc.dram_tensor("wih2T", [H, G], BF16, kind="ExternalInput").ap(),
    ]
    whh = [
        nc.dram_tensor(f"whh{i}T", [H, G], BF16, kind="ExternalInput").ap()
        for i in range(3)
    ]
    # bias_bc[l]: [128, G] broadcast of (bih + [bhh_rz, 0]) -- added to gx.
    bias_bc = [
        nc.dram_tensor(f"bias{i}", [128, G], BF16, kind="ExternalInput").ap()
        for i in range(3)
    ]
    # misc[l]: [3, 1088] = per-layer bhh_n (1024) ++ ones (64)
    misc_d = nc.dram_tensor("misc", [3, 1088], BF16,
                            kind="ExternalInput").ap()
    gamma_pm = nc.dram_tensor("gamma_pm", [128, KH], F32, kind="ExternalInput").ap()
    beta_pm = nc.dram_tensor("beta_pm", [128, KH], F32, kind="ExternalInput").ap()
    fcw_pm = nc.dram_tensor("fcw_pm", [128, KH], F32, kind="ExternalInput").ap()
    fcb_d = nc.dram_tensor("fcb", [1, 1], F32, kind="ExternalInput").ap()
    # const_d: [eye(128) | ones(128x64) | zeros(128x64)]
    const_d = nc.dram_tensor("const_d", [128, 256], BF16,
                             kind="ExternalInput").ap()

    gxa_d = nc.dram_tensor("gxa_d", [T_ * BL, G], BF16).ap()
    gxb_d = nc.dram_tensor("gxb_d", [T_ * BL, G], BF16).ap()
    bn_in = nc.dram_tensor("bn_in", [2 * H], F32).ap()
    bn_out = nc.dram_tensor("bn_out", [2 * H], F32, addr_space="Shared").ap()
    out_d = nc.dram_tensor("out", [BL], F32, kind="ExternalOutput").ap()

    const_pool = tc.alloc_tile_pool(name="const", bufs=1)
    constt = const_pool.tile([128, 256], BF16, name="constt")
    nc.sync.dma_start(constt[:], const_d[:])
    ident = constt[:, 0:128]
    zeros64 = constt[:, 192:256]

    whhp = tc.alloc_tile_pool(name="whhp", bufs=KH)
    wihp = tc.alloc_tile_pool(name="wihp", bufs=KH)
    biasp = tc.alloc_tile_pool(name="biasp", bufs=1)
    miscp = tc.alloc_tile_pool(name="miscp", bufs=3)
    pairp = tc.alloc_tile_pool(name="pairp", bufs=2)
    hzp = tc.alloc_tile_pool(name="hzp", bufs=1)
    # small PSUM pool shared by A-phase chunks and PE transposes
    smallp = tc.alloc_tile_pool(name="smallp", bufs=2, space="PSUM")
    stp = tc.alloc_tile_pool(name="stp", bufs=2)

    # persistent hT pair tiles: one [128, KH*128] tile per set,
    # chunk k at columns [k*128, (k+1)*128), timestep half at +hf*64
    pair = [pairp.tile([128, KH * 128], BF16, name=f"pair{s}")
            for s in range(2)]
    h_zero = hzp.tile([BL, H], BF16, name="h_zero")
    nc.gpsimd.memset(h_zero[:], 0.0)

    misc_t = []
    for i in range(3):
        mt = miscp.tile([1, 1088], BF16, name=f"misc_l{i}", tag="misc")
        nc.sync.dma_start(mt[:], misc_d[i:i + 1, :])
        misc_t.append(mt)

    def load_w(pool, src, kk, tag):
        tiles = []
        for k in range(kk):
            wt = pool.tile([128, G], BF16, name=f"{tag}_{k}",
                           tag="wih" if pool is wihp else "whh")
            if kk == 1:
                nc.sync.dma_start(wt[:F, :], src[:])
            else:
                nc.sync.dma_start(wt[:], src[k * 128:(k + 1) * 128, :])
            tiles.append(wt)
        return tiles

    def load_bias(layer):
        bt = biasp.tile([128, G], BF16, name=f"bias_l{layer}", tag="bias")
        nc.sync.dma_start(bt[:], bias_bc[layer][:])
        return bt

    NA = 12           # A-phase psum chunks per 128-token block
    AW = G // NA      # 256

    def emit_a_chunks(lhs_aps, kk, wt_tiles, bias_t, gx_dst, row0, chunks):
        """One half (or all) of a phase-A block: psum chunks -> bias add ->
        one DMA store per contiguous half."""
        st = stp.tile([128, G // 2], BF16, name=f"ast_{row0}_{chunks[0]}",
                      tag="ast")
        for ci, c in enumerate(chunks):
            csl = slice(c * AW, (c + 1) * AW)
            ps = smallp.tile([128, AW], F32, name=f"aps_{row0}_{c}", tag="ps")
            for k in range(kk):
                rhs = wt_tiles[k][:F, csl] if kk == 1 else wt_tiles[k][:, csl]
                nc.tensor.matmul(ps[:], lhs_aps[k], rhs,
                                 start=(k == 0), stop=(k == kk - 1))
            nc.vector.tensor_tensor(st[:, ci * AW:(ci + 1) * AW], ps[:],
                                    bias_t[:, csl], AOP.add)
        c0 = chunks[0] * AW
        nc.sync.dma_start(
            gx_dst[row0:row0 + 128, c0:c0 + G // 2], st[:])

    # ---- Phase A0: gx0 = x @ Wih0T + bias0, written to gxa_d ----
    wih0_t = load_w(wihp, wih[0], 1, "wih0")
    bias0_t = load_bias(0)
    with tc.tile_pool(name="a0lhs", bufs=4) as a0lhs:
        for j in range(nblocks):
            lt = a0lhs.tile([F, 128], BF16, name=f"a0lhs_{j}", tag="a0l")
            nc.sync.dma_start(
                lt[:], xT[:, 2 * j:2 * j + 2, :].rearrange("f t b -> f (t b)"))
            emit_a_chunks([lt[:]] , 1, wih0_t, bias0_t, gxa_d, j * 128,
                          list(range(0, NA // 2)))
            emit_a_chunks([lt[:]], 1, wih0_t, bias0_t, gxa_d, j * 128,
                          list(range(NA // 2, NA)))

    # ---- Recurrence layers ----
    gx_src = [gxa_d, gxb_d, gxa_d]
    gx_dst = [gxb_d, gxa_d, None]

    def phase_r(layer, whh_tiles, wih_next, bias_next):
        src = gx_src[layer]
        dst = gx_dst[layer]
        mt = misc_t[layer]
        ones64 = mt[:, 1024:1088]
        h_prev = h_zero

        def a_half(j, ha):
            lhs = [pair[j % 2][:, k * 128:(k + 1) * 128] for k in range(KH)]
            emit_a_chunks(lhs, KH, wih_next, bias_next, dst, j * 128,
                          list(range(ha * (NA // 2), (ha + 1) * (NA // 2))))

        with (
            tc.tile_pool(name=f"gxp{layer}", bufs=2) as gxp,
            tc.tile_pool(name=f"sp{layer}", bufs=1) as sp,
            tc.tile_pool(name=f"np{layer}", bufs=1) as np_,
            tc.tile_pool(name=f"hp{layer}", bufs=2) as hp,
            tc.tile_pool(name=f"tmp{layer}", bufs=2) as tmp,
            tc.tile_pool(name=f"rzp{layer}", bufs=1, space="PSUM") as rzp,
            tc.tile_pool(name=f"nnp{layer}", bufs=1, space="PSUM") as nnp,
        ):
            def fetch(t):
                g = gxp.tile([BL, G], BF16, name=f"gx{layer}_{t}", tag="gx")
                nc.sync.dma_start(g[:], src[t * BL:(t + 1) * BL, :])
                return g

            gxt_tiles = {0: fetch(0)}
            for t in range(T_):
                if t + 1 < T_:
                    gxt_tiles[t + 1] = fetch(t + 1)
                gxt = gxt_tiles.pop(t)

                if t == 0:
                    htp = [zeros64 for _ in range(KH)]
                else:
                    q = (t - 1) // 2
                    hf = (t - 1) % 2
                    htp = [pair[q % 2][:, k * 128 + hf * 64:
                                      k * 128 + hf * 64 + 64]
                           for k in range(KH)]

                rz = rzp.tile([BL, 2048], F32, name=f"rz_{t}", tag="rz")
                nn = nnp.tile([BL, 1024], F32, name=f"nn_{t}", tag="nn")
                # k-grouped: k0-3 (gated only on the half-0 pair copy) for all
                # chunks first, then k4-7 with the r/z chunks leading so r-add
                # unblocks as early as possible after the half-1 copy lands
                for n in range(2):
                    nsl = slice(n * 512, (n + 1) * 512)
                    nc.tensor.matmul(nn[:, nsl], ones64[:, :BL],
                                     mt[:, n * 512:(n + 1) * 512],
                                     start=True, stop=False)
                for kg, krange in ((0, range(0, KH // 2)),
                                   (1, range(KH // 2, KH))):
                    for n in range(4):
                        nsl = slice(n * 512, (n + 1) * 512)
                        for k in krange:
                            nc.tensor.matmul(rz[:, nsl], htp[k],
                                             whh_tiles[k][:, nsl],
                                             start=(k == 0),
                                             stop=(k == KH - 1))
                    for n in range(2):
                        nsl = slice(n * 512, (n + 1) * 512)
                        gsl = slice(2048 + n * 512, 2048 + (n + 1) * 512)
                        for k in krange:
                            nc.tensor.matmul(nn[:, nsl], htp[k],
                                             whh_tiles[k][:, gsl],
                                             start=False, stop=(k == KH - 1))

                # interleaved phase-A half block for layer+1 (PE filler
                # during this step's elementwise chain)
                if wih_next is not None and t >= 2:
                    a_half(t // 2 - 1, t % 2)

                # r = sigmoid(gx_r + gh_r); z = sigmoid(gx_z + gh_z)
                s = sp.tile([BL, 2048], BF16, name=f"s_{t}", tag="s")
                nc.vector.tensor_tensor(s[:, 0:H], gxt[:, 0:H], rz[:, 0:H],
                                        AOP.add)
                nc.scalar.activation(s[:, 0:H], s[:, 0:H], ACTF.Sigmoid)
                nc.vector.tensor_tensor(s[:, H:2 * H], gxt[:, H:2 * H],
                                        rz[:, H:2 * H], AOP.add)
                nc.scalar.activation(s[:, H:2 * H], s[:, H:2 * H],
                                     ACTF.Sigmoid)

                nt = np_.tile([BL, H], BF16, name=f"n_{t}", tag="n")
                h_new = hp.tile([BL, H], BF16, name=f"h_{t}", tag="h")
                HH = H // 2
                for hf2 in range(2):
                    hs = slice(hf2 * HH, (hf2 + 1) * HH)
                    gn = slice(2 * H + hf2 * HH, 2 * H + (hf2 + 1) * HH)
                    t1 = tmp.tile([BL, HH], BF16, name=f"t1_{t}_{hf2}",
                                  tag="tmp")
                    nc.vector.tensor_tensor(t1[:], s[:, hs], nn[:, hs],
                                            AOP.mult)
                    t2 = tmp.tile([BL, HH], BF16, name=f"t2_{t}_{hf2}",
                                  tag="tmp")
                    nc.vector.tensor_tensor(t2[:], gxt[:, gn], t1[:], AOP.add)
                    nc.scalar.activation(nt[:, hs], t2[:], ACTF.Tanh)
                    # h = n + z * (h_prev - n)
                    t3 = tmp.tile([BL, HH], BF16, name=f"t3_{t}_{hf2}",
                                  tag="tmp")
                    nc.vector.tensor_tensor(t3[:], h_prev[:, hs], nt[:, hs],
                                            AOP.subtract)
                    t4 = tmp.tile([BL, HH], BF16, name=f"t4_{t}_{hf2}",
                                  tag="tmp")
                    nc.vector.tensor_tensor(
                        t4[:], s[:, H + hf2 * HH:H + (hf2 + 1) * HH], t3[:],
                        AOP.mult)
                    nc.vector.tensor_tensor(h_new[:, hs], nt[:, hs], t4[:],
                                            AOP.add)
                    tp = smallp.tile([128, 4 * BL], BF16,
                                     name=f"tr_{t}_{hf2}", tag="ps")
                    for ki, k in enumerate(
                            range(hf2 * KH // 2, (hf2 + 1) * KH // 2)):
                        nc.tensor.transpose(
                            tp[:, ki * BL:(ki + 1) * BL],
                            h_new[:, k * 128:(k + 1) * 128],
                            ident[0:BL, 0:BL])
                    k0 = hf2 * KH // 2
                    dest = pair[(t // 2) % 2].rearrange(
                        "p (k c) -> p k c", c=128)[
                        :, k0:k0 + 4,
                        (t % 2) * 64:(t % 2) * 64 + 64]
                    if hf2 == 0:
                        nc.scalar.copy(dest, tp[:])
                    else:
                        nc.vector.tensor_copy(dest, tp[:])
                h_prev = h_new

            # trailing phase-A half blocks (last pair)
            if wih_next is not None:
                a_half(nblocks - 1, 0)
                a_half(nblocks - 1, 1)
        return h_prev

    whh_tiles = load_w(whhp, whh[0], KH, "whh0")
    wih_next = load_w(wihp, wih[1], KH, "wih1")
    bias_next = load_bias(1)
    h_last = None
    for layer in range(3):
        h_last = phase_r(layer, whh_tiles, wih_next, bias_next)
        if layer < 2:
            whh_tiles = load_w(whhp, whh[layer + 1], KH, f"whh{layer + 1}")
            if layer < 1:
                wih_next = load_w(wihp, wih[2], KH, "wih2")
                bias_next = load_bias(2)
            else:
                wih_next = None
                bias_next = None

    # ---- BatchNorm stats + BN/FC folded head ----
    with (
        tc.tile_pool(name="bnps", bufs=1, space="PSUM") as bn_psum,
        tc.tile_pool(name="bnsb", bufs=1) as bn_sb,
    ):
        ones_col = constt[0:BL, 128:129]
        h_sq = bn_sb.tile([BL, H], BF16, name="h_sq")
        nc.scalar.activation(h_sq[:], h_last[:], ACTF.Square)

        stats_ps = bn_psum.tile([128, 2 * KH], F32, name="stats_ps", tag="bnp")
        for k in range(KH):
            ksl = slice(k * 128, (k + 1) * 128)
            nc.tensor.matmul(stats_ps[:, k:k + 1], h_last[:, ksl],
                             ones_col, start=True, stop=True)
            nc.tensor.matmul(stats_ps[:, KH + k:KH + k + 1], h_sq[:, ksl],
                             ones_col, start=True, stop=True)
        stats_sb = bn_sb.tile([128, 2 * KH], F32, name="stats_sb")
        nc.scalar.copy(stats_sb[:], stats_ps[:])
        nc.sync.dma_start(bn_in.rearrange("(p f) -> p f", p=128), stats_sb[:])
        nc.gpsimd.collective_compute(
            "AllReduce", AOP.add,
            replica_groups=[list(range(N_CORES))],
            ins=[bn_in[:]], outs=[bn_out[:]])
        agg = bn_sb.tile([128, 2 * KH], F32, name="agg")
        nc.sync.dma_start(agg[:], bn_out.rearrange("(p f) -> p f", p=128))

        gpm = bn_sb.tile([128, KH], F32, name="gpm")
        nc.sync.dma_start(gpm[:], gamma_pm[:])
        bpm = bn_sb.tile([128, KH], F32, name="bpm")
        nc.sync.dma_start(bpm[:], beta_pm[:])
        wpm = bn_sb.tile([128, KH], F32, name="wpm")
        nc.sync.dma_start(wpm[:], fcw_pm[:])
        fcb_t = bn_sb.tile([1, 1], F32, name="fcb_t")
        nc.sync.dma_start(fcb_t[:], fcb_d[:])

        mu = bn_sb.tile([128, KH], F32, name="mu")
        nc.scalar.mul(mu[:], agg[:, 0:KH], 1.0 / B)
        ex2 = bn_sb.tile([128, KH], F32, name="ex2")
        nc.scalar.mul(ex2[:], agg[:, KH:2 * KH], 1.0 / B)
        musq = bn_sb.tile([128, KH], F32, name="musq")
        nc.vector.tensor_tensor(musq[:], mu[:], mu[:], AOP.mult)
        var = bn_sb.tile([128, KH], F32, name="var")
        nc.vector.tensor_tensor(var[:], ex2[:], musq[:], AOP.subtract)
        eps_t = bn_sb.tile([128, 1], F32, name="eps_t")
        nc.gpsimd.memset(eps_t[:], EPS)
        std = bn_sb.tile([128, KH], F32, name="std")
        nc.scalar.activation(std[:], var[:], ACTF.Sqrt, bias=eps_t[:])
        rstd = bn_sb.tile([128, KH], F32, name="rstd")
        nc.vector.reciprocal(rstd[:], std[:])
        scoef = bn_sb.tile([128, KH], F32, name="scoef")
        nc.vector.tensor_tensor(scoef[:], rstd[:], gpm[:], AOP.mult)
        sw = bn_sb.tile([128, KH], F32, name="sw")
        nc.vector.tensor_tensor(sw[:], scoef[:], wpm[:], AOP.mult)
        sw_bf = bn_sb.tile([128, KH], BF16, name="sw_bf")
        nc.scalar.copy(sw_bf[:], sw[:])
        ms = bn_sb.tile([128, KH], F32, name="ms")
        nc.vector.tensor_tensor(ms[:], mu[:], scoef[:], AOP.mult)
        d = bn_sb.tile([128, KH], F32, name="d")
        nc.vector.tensor_tensor(d[:], bpm[:], ms[:], AOP.subtract)
        dw = bn_sb.tile([128, KH], F32, name="dw")
        nc.vector.tensor_tensor(dw[:], d[:], wpm[:], AOP.mult)
        dw1 = bn_sb.tile([128, 1], F32, name="dw1")
        nc.vector.reduce_sum(dw1[:], dw[:], mybir.AxisListType.X)
        dw1_bf = bn_sb.tile([128, 1], BF16, name="dw1_bf")
        nc.scalar.copy(dw1_bf[:], dw1[:])
        ones128 = constt[:, 128:129]
        c_ps = bn_psum.tile([1, 1], F32, name="c_ps", tag="bnc")
        nc.tensor.matmul(c_ps[:], dw1_bf[:], ones128, start=True, stop=True)
        c_sb = bn_sb.tile([1, 1], BF16, name="c_sb")
        nc.vector.tensor_tensor(c_sb[:], c_ps[:], fcb_t[:], AOP.add)

        # y = h63 @ sw + C   via hT63 chunks from the last pair slot
        q63 = ((T_ - 1) // 2) % 2
        h63 = (T_ - 1) % 2
        y_ps = bn_psum.tile([BL, 1], F32, name="y_ps", tag="bny")
        for k in range(KH):
            nc.tensor.matmul(
                y_ps[:],
                pair[q63][:, k * 128 + h63 * 64:k * 128 + h63 * 64 + 64],
                sw_bf[:, k:k + 1],
                start=(k == 0), stop=False)
        onesb = constt[0:1, 128:128 + BL]
        nc.tensor.matmul(y_ps[:], onesb, c_sb[:], start=False, stop=True)
        res = bn_sb.tile([BL, 1], F32, name="res")
        nc.scalar.activation(res[:], y_ps[:], ACTF.Sigmoid)
        nc.sync.dma_start(out_d.rearrange("(p f) -> p f", f=1), res[:])

    stp.release()
    smallp.release()
    hzp.release()
    pairp.release()
    miscp.release()
    biasp.release()
    wihp.release()
    whhp.release()
    const_pool.release()


_PROGRAM_CACHE = {}


def build_program(seq_len=T):
    if seq_len in _PROGRAM_CACHE:
        return _PROGRAM_CACHE[seq_len]
    nc = bacc.Bacc("TRN2", target_bir_lowering=False, debug=False,
                   num_devices=N_CORES)
    with nc.allow_low_precision(reason="bf16 operands are intentional"):
        with tile.TileContext(nc) as tc:
            _emit(nc, tc, seq_len)
    nc.compile()
    _PROGRAM_CACHE[seq_len] = nc
    return nc


def make_in_maps(inputs, seq_len=T):
    f32 = np.float32
    bf = ml_dtypes.bfloat16

    def prep_shared():
        m = {}
        misc_all = np.zeros((3, 1088), dtype=f32)
        m["wih0T"] = np.ascontiguousarray(inputs["Wih0"].T).astype(bf)
        m["wih1T"] = np.ascontiguousarray(inputs["Wih1"].T).astype(bf)
        m["wih2T"] = np.ascontiguousarray(inputs["Wih2"].T).astype(bf)
        for i in range(3):
            m[f"whh{i}T"] = np.ascontiguousarray(
                inputs[f"Whh{i}"].T).astype(bf)
            bih = np.asarray(inputs[f"bih{i}"], dtype=f32)
            bhh = np.asarray(inputs[f"bhh{i}"], dtype=f32)
            bias = bih.copy()
            bias[:2 * H] += bhh[:2 * H]
            m[f"bias{i}"] = np.ascontiguousarray(
                np.broadcast_to(bias, (128, G))).astype(bf)
            misc_all[i, :H] = bhh[2 * H:]
            misc_all[i, H:H + 64] = 1.0
        m["misc"] = misc_all.astype(bf)
        for name, key in (("gamma_pm", "gamma"), ("beta_pm", "beta")):
            v = np.asarray(inputs[key], dtype=f32)
            m[name] = np.ascontiguousarray(v.reshape(KH, 128).T)
        fcw = np.asarray(inputs["fcW"], dtype=f32).reshape(H)
        m["fcw_pm"] = np.ascontiguousarray(fcw.reshape(KH, 128).T)
        m["fcb"] = np.asarray(inputs["fcb"], dtype=f32).reshape(1, 1)
        cd = np.zeros((128, 256), dtype=f32)
        cd[:, :128] = np.eye(128, dtype=f32)
        cd[:, 128:192] = 1.0
        m["const_d"] = cd.astype(bf)
        return m

    shared = prep_shared()
    x = np.asarray(inputs["x"], dtype=f32)
    in_maps = []
    for c in range(N_CORES):
        xs = x[c * BL:(c + 1) * BL, :seq_len, :]            # [BL, T, F]
        xT_c = np.ascontiguousarray(xs.transpose(2, 1, 0))  # [F, T, BL]
        m = dict(shared)
        m["xT"] = xT_c.astype(bf)
        in_maps.append(m)
    return in_maps


def kernel(**inputs):
    nc = build_program(T)
    in_maps = make_in_maps(inputs, T)
    res = run_bass_kernel_spmd(nc, in_maps, list(range(N_CORES)))
    out = np.concatenate([res.results[c]["out"] for c in range(N_CORES)])
    return out.astype(np.float32)
